# revision 13
# baseline (speedup 1.0000x reference)
"""Trainium2 Bass kernel for nn_EncodingLayer (dense transformer encoder layer).

Reference computation (B=2, S=2048, H=128, NH=8):
    Q/K/V = per-head full-dim projections of x, scores = QK^T/sqrt(H),
    A = softmax(scores), o = A@V, concat heads, y = o@Wo+bo,
    y = LN1(y), f = relu(relu(y@W1+b1)@W2+b2), out = LN2(y+f).

Sharding: data-parallel over query rows. Core c (of 8) owns batch b=c//4 and
query rows q0=(c%4)*512 .. q0+512 of that batch. Each core computes K/V for
its full batch (4x replicated compute, tiny) and the full epilogue for its
512 rows. No collectives; host concatenates the 8 [512,128] slices.

Within a core the attention runs in "transposed score" layout:
    QT/KT = [e, s] via PE, scores^T[t,s] chunks on PE (fp32r, full rate),
    P^T = exp(scores^T) on ACT straight out of PSUM, o^T accumulated on PE
    with V[t,e] chunks as stationary, softmax denominator via ones-vector
    matmul (sum over t = partition dim), division applied to o^T (tiny).
Since |scores| < ~0.4 for this problem scale, softmax without max-subtraction
is numerically exact; bv folds into o^T after division because softmax rows
sum to one.
"""

import math
import numpy as np
from contextlib import ExitStack

import concourse.bass as bass
import concourse.bacc as bacc
import concourse.mybir as mybir
import concourse.tile as tile
from concourse.bass_utils import run_bass_kernel_spmd
from concourse.masks import make_identity

B, S, H, NH = 2, 2048, 128, 8
F = 2 * H                      # FFN hidden dim (256)
NCORES = 8
SQ = (B * S) // NCORES         # 512 query rows per core
TC = S // 128                  # 16 key/value chunks of 128
LN_EPS = 1e-5
FP32 = mybir.dt.float32
FP32R = mybir.dt.float32r
AF = mybir.ActivationFunctionType
ALU = mybir.AluOpType


def _r(ap):
    return ap.bitcast(FP32R)


def _bcast_ap(ap, parts):
    """Partition-broadcast view of a single-partition AP (for DMA)."""
    return bass.AP(tensor=ap.tensor, offset=ap.offset, ap=[[0, parts]] + list(ap.ap)[1:])


def _ln_tile(nc, pool, out_ap, in_ap, eps_tile, g_bc, beta_bc):
    """LayerNorm over the free dim of a [128, H] tile: out = (x-m)/sqrt(v+eps)*g+b."""
    stats = pool.tile([128, nc.vector.BN_STATS_DIM], FP32, tag="ln_stats")
    nc.vector.bn_stats(out=stats[:], in_=in_ap)
    mv = pool.tile([128, nc.vector.BN_AGGR_DIM], FP32, tag="ln_mv")
    nc.vector.bn_aggr(out=mv[:], in_=stats[:])
    std = pool.tile([128, 1], FP32, tag="ln_std")
    nc.scalar.activation(out=std[:], in_=mv[:, 1:2], func=AF.Sqrt, bias=eps_tile[:])
    nc.vector.reciprocal(out=std[:], in_=std[:])
    tmp = pool.tile([128, H], FP32, tag="ln_tmp")
    nc.vector.tensor_scalar(
        out=tmp[:], in0=in_ap, scalar1=mv[:, 0:1], scalar2=std[:],
        op0=ALU.subtract, op1=ALU.mult,
    )
    nc.vector.tensor_mul(out=tmp[:], in0=tmp[:], in1=g_bc[:])
    nc.vector.tensor_add(out=out_ap, in0=tmp[:], in1=beta_bc[:])


def build_module():
    nc = bacc.Bacc(None)

    xb_d = nc.declare_dram_parameter("xb", [S, H], FP32, isOutput=False)
    xq_d = nc.declare_dram_parameter("xq", [SQ, H], FP32, isOutput=False)
    wq_d = nc.declare_dram_parameter("wq", [NH, H, H], FP32, isOutput=False)
    bq_d = nc.declare_dram_parameter("bq", [NH, H], FP32, isOutput=False)
    wk_d = nc.declare_dram_parameter("wk", [NH, H, H], FP32, isOutput=False)
    bk_d = nc.declare_dram_parameter("bk", [NH, H], FP32, isOutput=False)
    wv_d = nc.declare_dram_parameter("wv", [NH, H, H], FP32, isOutput=False)
    bv_d = nc.declare_dram_parameter("bv", [NH, H], FP32, isOutput=False)
    wo_d = nc.declare_dram_parameter("wo", [NH * H, H], FP32, isOutput=False)
    bo_d = nc.declare_dram_parameter("bo", [H], FP32, isOutput=False)
    w1_d = nc.declare_dram_parameter("w1", [H, F], FP32, isOutput=False)
    b1_d = nc.declare_dram_parameter("b1", [F], FP32, isOutput=False)
    w2_d = nc.declare_dram_parameter("w2", [F, H], FP32, isOutput=False)
    b2_d = nc.declare_dram_parameter("b2", [H], FP32, isOutput=False)
    g1_d = nc.declare_dram_parameter("g1", [H], FP32, isOutput=False)
    be1_d = nc.declare_dram_parameter("beta1", [H], FP32, isOutput=False)
    g2_d = nc.declare_dram_parameter("g2", [H], FP32, isOutput=False)
    be2_d = nc.declare_dram_parameter("beta2", [H], FP32, isOutput=False)
    out_d = nc.declare_dram_parameter("out", [SQ, H], FP32, isOutput=True)

    with tile.TileContext(nc) as tc, ExitStack() as ctx:
        singles = ctx.enter_context(tc.tile_pool(name="singles", bufs=1))
        work = ctx.enter_context(tc.tile_pool(name="work", bufs=3))

        # ---- constants / weights to SBUF ----
        ident = singles.tile([128, 128], FP32)
        make_identity(nc, ident[:])
        ones_st = singles.tile([128, 1], FP32)
        nc.vector.memset(ones_st[:], 1.0)
        ones_col = singles.tile([128, 1], FP32)   # lhsT for partition sums
        nc.vector.tensor_copy(out=_r(ones_col[:]), in_=ones_st[:])
        ones_row = singles.tile([1, 128], FP32)   # lhsT for partition broadcast
        nc.vector.memset(ones_row[:], 1.0)
        eps_t = singles.tile([128, 1], FP32)
        nc.vector.memset(eps_t[:], LN_EPS)

        wq_sb = singles.tile([H, NH, H], FP32)    # (d, h, e)
        nc.gpsimd.dma_start(out=_r(wq_sb[:]), in_=wq_d[:].rearrange("h d e -> d h e"))
        wk_sb = singles.tile([H, NH, H], FP32)
        nc.gpsimd.dma_start(out=_r(wk_sb[:]), in_=wk_d[:].rearrange("h d e -> d h e"))
        wv_sb = singles.tile([H, NH, H], FP32)
        nc.gpsimd.dma_start(out=_r(wv_sb[:]), in_=wv_d[:].rearrange("h d e -> d h e"))
        wo_sb = singles.tile([H, NH, H], FP32)    # (e, h, j)
        nc.gpsimd.dma_start(out=_r(wo_sb[:]), in_=wo_d[:].rearrange("(h e) j -> e h j", h=NH))
        w1_sb = singles.tile([H, F], FP32)        # (d, f)
        nc.gpsimd.dma_start(out=_r(w1_sb[:]), in_=w1_d[:])
        w2_sb = singles.tile([H, 2, H], FP32)     # (f%128, f//128, j)
        nc.gpsimd.dma_start(out=_r(w2_sb[:]), in_=w2_d[:].rearrange("(c f) j -> f c j", c=2))

        bq_sb = singles.tile([H, NH], FP32)       # (e, h)
        nc.sync.dma_start(out=bq_sb[:], in_=bq_d[:].rearrange("h e -> e h"))
        bk_sb = singles.tile([H, NH], FP32)
        nc.sync.dma_start(out=bk_sb[:], in_=bk_d[:].rearrange("h e -> e h"))
        bv_sb = singles.tile([H, NH], FP32)
        nc.sync.dma_start(out=bv_sb[:], in_=bv_d[:].rearrange("h e -> e h"))
        bo_sb = singles.tile([H, 1], FP32)        # per-partition (j)
        nc.sync.dma_start(out=bo_sb[:], in_=bo_d[:].rearrange("(j o) -> j o", o=1))
        b1_sb = singles.tile([H, 2], FP32)        # (f%128, f//128)
        nc.sync.dma_start(out=b1_sb[:], in_=b1_d[:].rearrange("(c f) -> f c", c=2))
        b2_sb = singles.tile([H, 1], FP32)
        nc.sync.dma_start(out=b2_sb[:], in_=b2_d[:].rearrange("(j o) -> j o", o=1))

        g1_bc = singles.tile([128, H], FP32)      # free-dim vectors broadcast over partitions
        nc.sync.dma_start(out=g1_bc[:], in_=_bcast_ap(g1_d[:].rearrange("(o j) -> o j", o=1), 128))
        be1_bc = singles.tile([128, H], FP32)
        nc.sync.dma_start(out=be1_bc[:], in_=_bcast_ap(be1_d[:].rearrange("(o j) -> o j", o=1), 128))
        g2_bc = singles.tile([128, H], FP32)
        nc.sync.dma_start(out=g2_bc[:], in_=_bcast_ap(g2_d[:].rearrange("(o j) -> o j", o=1), 128))
        be2_bc = singles.tile([128, H], FP32)
        nc.sync.dma_start(out=be2_bc[:], in_=_bcast_ap(be2_d[:].rearrange("(o j) -> o j", o=1), 128))

        # ---- x into SBUF + transposes xT=[d, S], xqT=[d, SQ] ----
        xb_sb = singles.tile([128, TC, H], FP32)  # (s%128, sc, d)
        nc.sync.dma_start(out=xb_sb[:], in_=xb_d[:].rearrange("(sc p) d -> p sc d", p=128))
        xq_sb = singles.tile([128, SQ // 128, H], FP32)
        nc.sync.dma_start(out=xq_sb[:], in_=xq_d[:].rearrange("(sc p) d -> p sc d", p=128))
        xT = singles.tile([H, S], FP32)
        xqT = singles.tile([H, SQ], FP32)

        # PE matmuls (fused LDWEIGHTS) can carry only ONE semaphore wait in
        # codegen. Each dummy transpose below makes PE observe one DMA/engine
        # semaphore so no later matmul needs to wait on two at once; _zd()
        # writes a [1,1] dummy into a new PSUM pool's first tile so the
        # pool-transition (released-zone) dependency is absorbed there
        # instead of landing on a real matmul that also has a data wait.
        def _zd(tile_ap):
            nc.tensor.matmul(tile_ap[0:1, 0:1], ident[:, 0:1], ident[:, 0:1],
                             start=True, stop=True)

        with tc.tile_pool(name="abs_ps", bufs=7, space="PSUM") as abs_ps:
            for absorber in (
                ident[:], xb_sb[:, 0, :], wv_sb[:, 0, :].bitcast(FP32),
                wq_sb[:, 0, :].bitcast(FP32), wo_sb[:, 0, :].bitcast(FP32),
                w1_sb[:, 0:128].bitcast(FP32), w2_sb[:, 0, :].bitcast(FP32),
            ):
                pt = abs_ps.tile([128, 128], FP32, tag="abs")
                nc.tensor.transpose(pt[:], absorber, ident[:])

        with tc.tile_pool(name="tp_ps", bufs=2, space="PSUM") as tp_ps:
            for sc in range(SQ // 128):
                pt = tp_ps.tile([128, 128], FP32, tag="tp")
                if sc == 0:
                    _zd(pt)
                nc.tensor.transpose(pt[:], xq_sb[:, sc, :], ident[:])
                nc.vector.tensor_copy(out=_r(xqT[:, sc * 128:(sc + 1) * 128]), in_=pt[:])
            for sc in range(TC):
                pt = tp_ps.tile([128, 128], FP32, tag="tp")
                nc.tensor.transpose(pt[:], xb_sb[:, sc, :], ident[:])
                nc.vector.tensor_copy(out=_r(xT[:, sc * 128:(sc + 1) * 128]), in_=pt[:])

        # ---- V for all heads: v_sb[t%128, tc, h, e] = (x @ Wv)[t, (h e)] ----
        v_sb = singles.tile([128, TC, NH, H], FP32)
        with tc.tile_pool(name="v_ps", bufs=2, space="PSUM") as v_ps:
            for tcc in range(TC):
                vp = v_ps.tile([128, NH * H], FP32, tag="v")
                if tcc == 0:
                    _zd(vp)
                for half in range(2):
                    nc.tensor.matmul(
                        vp[:, half * 512:(half + 1) * 512],
                        _r(xT[:, tcc * 128:(tcc + 1) * 128]),
                        _r(wv_sb[:, half * 4:(half + 1) * 4, :]),
                        start=True, stop=True,
                    )
                nc.vector.tensor_copy(out=_r(v_sb[:, tcc, :, :]), in_=vp[:])

        # ---- attention head loop ----
        kt_pool = ctx.enter_context(tc.tile_pool(name="kt", bufs=2))
        qt_pool = ctx.enter_context(tc.tile_pool(name="qt", bufs=2))
        pt_pool = ctx.enter_context(tc.tile_pool(name="pt", bufs=3))
        ot_pool = ctx.enter_context(tc.tile_pool(name="ot", bufs=2))

        yT_sb = singles.tile([H, SQ], FP32)  # attention block output (pre-LN), [j, s]

        with (
            tc.tile_pool(name="s_ps", bufs=2, space="PSUM") as s_ps,
            tc.tile_pool(name="o_ps", bufs=1, space="PSUM") as o_ps,
            tc.tile_pool(name="d_ps", bufs=1, space="PSUM") as d_ps,
            tc.tile_pool(name="b_ps", bufs=1, space="PSUM") as b_ps,
            tc.tile_pool(name="y_ps", bufs=1, space="PSUM") as y_ps,
        ):
            y_acc = y_ps.tile([H, SQ], FP32)
            _zd(y_acc)

            for h in range(NH):
                # K^T[e, t] and Q^T[e, s] with biases (and 1/sqrt(H) folded into Q)
                kt = kt_pool.tile([H, S], FP32, tag="kt")
                for i in range(S // 512):
                    kp = s_ps.tile([128, 1024], FP32, tag="s")
                    if h == 0 and i == 0:
                        _zd(kp)
                    nc.tensor.matmul(
                        kp[:, 0:512], _r(wk_sb[:, h, :]), _r(xT[:, i * 512:(i + 1) * 512]),
                        start=True, stop=True,
                    )
                    nc.vector.tensor_scalar_add(
                        out=_r(kt[:, i * 512:(i + 1) * 512]), in0=kp[:, 0:512],
                        scalar1=bk_sb[:, h:h + 1],
                    )
                qt = qt_pool.tile([H, SQ], FP32, tag="qt")
                qp = s_ps.tile([128, 1024], FP32, tag="s")
                nc.tensor.matmul(qp[:, 0:512], _r(wq_sb[:, h, :]), _r(xqT[:]),
                                 start=True, stop=True)
                nc.vector.tensor_scalar(
                    out=_r(qt[:]), in0=qp[:, 0:512], scalar1=bq_sb[:, h:h + 1],
                    scalar2=1.0 / math.sqrt(H), op0=ALU.add, op1=ALU.mult,
                )

                o_acc = o_ps.tile([H, SQ], FP32, tag="o")
                d_acc = d_ps.tile([1, SQ], FP32, tag="d")
                if h == 0:
                    _zd(o_acc)
                    _zd(d_acc)

                for g in range(TC // 2):  # pairs of t-chunks share one 2-bank psum tile
                    sp = s_ps.tile([128, 1024], FP32, tag="s")
                    for j in range(2):
                        tcc = 2 * g + j
                        nc.tensor.matmul(
                            sp[:, j * 512:(j + 1) * 512],
                            _r(kt[:, tcc * 128:(tcc + 1) * 128]), _r(qt[:]),
                            start=True, stop=True,
                        )
                    pt = pt_pool.tile([128, 1024], FP32, tag="pt")
                    nc.scalar.activation(out=_r(pt[:]), in_=sp[:], func=AF.Exp)
                    for j in range(2):
                        tcc = 2 * g + j
                        nc.tensor.matmul(
                            d_acc[:], _r(ones_col[:]), _r(pt[:, j * 512:(j + 1) * 512]),
                            start=(tcc == 0), stop=(tcc == TC - 1),
                        )
                        nc.tensor.matmul(
                            o_acc[:], _r(v_sb[:, tcc, h, :]), _r(pt[:, j * 512:(j + 1) * 512]),
                            start=(tcc == 0), stop=(tcc == TC - 1),
                        )

                # softmax denominators -> reciprocal -> broadcast over partitions
                rec = ot_pool.tile([1, SQ], FP32, tag="rec")
                nc.vector.reciprocal(out=rec[:], in_=d_acc[:])
                bc = b_ps.tile([128, SQ], FP32, tag="bc")
                if h == 0:
                    _zd(bc)
                nc.tensor.matmul(bc[:], ones_row[:], rec[:], start=True, stop=True)
                bc_sb = ot_pool.tile([128, SQ], FP32, tag="bc_sb")
                nc.vector.tensor_copy(out=bc_sb[:], in_=bc[:])

                # o^T = o_acc / denom + bv  (softmax rows sum to 1)
                oT = ot_pool.tile([H, SQ], FP32, tag="oT")
                nc.vector.tensor_mul(out=_r(oT[:]), in0=o_acc[:], in1=bc_sb[:])
                nc.vector.tensor_scalar_add(out=_r(oT[:]), in0=oT[:], scalar1=bv_sb[:, h:h + 1])

                # y^T[j, s] += Wo_h^T o^T
                nc.tensor.matmul(y_acc[:], _r(wo_sb[:, h, :]), _r(oT[:]),
                                 start=(h == 0), stop=(h == NH - 1))

            nc.vector.tensor_scalar_add(out=yT_sb[:], in0=y_acc[:], scalar1=bo_sb[:])

        # ---- epilogue: transpose y, LN1, FFN (transposed), residual, LN2 ----
        y1_sb = singles.tile([128, SQ // 128, H], FP32)   # LN1 output, natural (s, j)
        y1T = singles.tile([H, SQ], FP32)                 # LN1 output, [d, s]
        out_sb = singles.tile([128, SQ // 128, H], FP32)

        with (
            tc.tile_pool(name="e_ps", bufs=2, space="PSUM") as e_ps,
            tc.tile_pool(name="u_ps", bufs=2, space="PSUM") as u_ps,
            tc.tile_pool(name="z_ps", bufs=1, space="PSUM") as z_ps,
        ):
            for sc in range(SQ // 128):
                yp = e_ps.tile([128, 128], FP32, tag="e")
                if sc == 0:
                    _zd(yp)
                nc.tensor.transpose(yp[:], yT_sb[:, sc * 128:(sc + 1) * 128], ident[:])
                _ln_tile(nc, work, y1_sb[:, sc, :], yp[:], eps_t, g1_bc, be1_bc)
            for sc in range(SQ // 128):
                yp = e_ps.tile([128, 128], FP32, tag="e")
                nc.tensor.transpose(yp[:], y1_sb[:, sc, :], ident[:])
                nc.vector.tensor_copy(out=_r(y1T[:, sc * 128:(sc + 1) * 128]), in_=yp[:])

            # u^T[f, s] = relu(W1^T y1 + b1), f in two 128-chunks
            uT = work.tile([H, 2, SQ], FP32, tag="uT")
            for fc in range(2):
                up = u_ps.tile([128, SQ], FP32, tag="u")
                if fc == 0:
                    _zd(up)
                nc.tensor.matmul(up[:], _r(w1_sb[:, fc * 128:(fc + 1) * 128]), _r(y1T[:]),
                                 start=True, stop=True)
                nc.scalar.activation(out=_r(uT[:, fc, :]), in_=up[:], func=AF.Relu,
                                     bias=b1_sb[:, fc:fc + 1])
            # z^T[j, s] = relu(W2^T u + b2)
            zp = z_ps.tile([H, SQ], FP32, tag="z")
            _zd(zp)
            for fc in range(2):
                nc.tensor.matmul(zp[:], _r(w2_sb[:, fc, :]), _r(uT[:, fc, :]),
                                 start=(fc == 0), stop=(fc == 1))
            zT = work.tile([H, SQ], FP32, tag="zT")
            nc.scalar.activation(out=zT[:], in_=zp[:], func=AF.Relu, bias=b2_sb[:])

            # residual + LN2, back in natural layout
            for sc in range(SQ // 128):
                rp = e_ps.tile([128, 128], FP32, tag="e")
                nc.tensor.transpose(rp[:], zT[:, sc * 128:(sc + 1) * 128], ident[:])
                r_sb = work.tile([128, H], FP32, tag="r_sb")
                nc.vector.tensor_add(out=r_sb[:], in0=rp[:], in1=y1_sb[:, sc, :])
                _ln_tile(nc, work, out_sb[:, sc, :], r_sb[:], eps_t, g2_bc, be2_bc)

        nc.sync.dma_start(out=out_d[:].rearrange("(sc p) j -> p sc j", p=128), in_=out_sb[:])

    nc.finalize()
    return nc


_CACHE: dict = {}


def _get_nc():
    if "nc" not in _CACHE:
        _CACHE["nc"] = build_module()
    return _CACHE["nc"]


def _in_maps(inputs):
    f32 = lambda a: np.ascontiguousarray(np.asarray(a), dtype=np.float32)
    x = f32(inputs["x"])
    shared = {
        "wq": f32(inputs["Wq"]), "bq": f32(inputs["bq"]),
        "wk": f32(inputs["Wk"]), "bk": f32(inputs["bk"]),
        "wv": f32(inputs["Wv"]), "bv": f32(inputs["bv"]),
        "wo": f32(inputs["Wo"]), "bo": f32(inputs["bo"]),
        "w1": f32(inputs["W1"]), "b1": f32(inputs["b1"]),
        "w2": f32(inputs["W2"]), "b2": f32(inputs["b2"]),
        "g1": f32(inputs["g1"]), "beta1": f32(inputs["beta1"]),
        "g2": f32(inputs["g2"]), "beta2": f32(inputs["beta2"]),
    }
    maps = []
    for c in range(NCORES):
        b, qi = divmod(c, NCORES // B)
        q0 = qi * SQ
        maps.append({
            "xb": np.ascontiguousarray(x[b]),
            "xq": np.ascontiguousarray(x[b, q0:q0 + SQ]),
            **shared,
        })
    return maps


def run(inputs, **kwargs):
    nc = _get_nc()
    res = run_bass_kernel_spmd(nc, _in_maps(inputs), core_ids=list(range(NCORES)), **kwargs)
    parts = [res.results[c]["out"] for c in range(NCORES)]
    y = np.concatenate(parts, axis=0).reshape(B, S, H).astype(np.float32)
    return y, res


def kernel(**inputs) -> np.ndarray:
    y, _ = run(inputs)
    return y


# revision 14
# speedup vs baseline: 1.0257x; 1.0257x over previous
"""Trainium2 Bass kernel for nn_EncodingLayer (dense transformer encoder layer).

Reference computation (B=2, S=2048, H=128, NH=8):
    Q/K/V = per-head full-dim projections of x, scores = QK^T/sqrt(H),
    A = softmax(scores), o = A@V, concat heads, y = o@Wo+bo,
    y = LN1(y), f = relu(relu(y@W1+b1)@W2+b2), out = LN2(y+f).

Sharding: data-parallel over query rows. Core c (of 8) owns batch b=c//4 and
query rows q0=(c%4)*512 .. q0+512 of that batch. Each core computes K/V for
its full batch (4x replicated compute, tiny) and the full epilogue for its
512 rows. No collectives; host concatenates the 8 [512,128] slices.

Within a core the attention runs in "transposed score" layout:
    QT/KT = [e, s] via PE, scores^T[t,s] chunks on PE (fp32r, full rate),
    P^T = exp(scores^T) on ACT straight out of PSUM, o^T accumulated on PE
    with V[t,e] chunks as stationary, softmax denominator via ones-vector
    matmul (sum over t = partition dim), division applied to o^T (tiny).
Since |scores| < ~0.4 for this problem scale, softmax without max-subtraction
is numerically exact; bv folds into o^T after division because softmax rows
sum to one.
"""

import math
import numpy as np
from contextlib import ExitStack

import concourse.bass as bass
import concourse.bacc as bacc
import concourse.mybir as mybir
import concourse.tile as tile
from concourse.bass_utils import run_bass_kernel_spmd
from concourse.masks import make_identity

B, S, H, NH = 2, 2048, 128, 8
F = 2 * H                      # FFN hidden dim (256)
NCORES = 8
SQ = (B * S) // NCORES         # 512 query rows per core
TC = S // 128                  # 16 key/value chunks of 128
LN_EPS = 1e-5
FP32 = mybir.dt.float32
FP32R = mybir.dt.float32r
AF = mybir.ActivationFunctionType
ALU = mybir.AluOpType


def _r(ap):
    return ap.bitcast(FP32R)


def _bcast_ap(ap, parts):
    """Partition-broadcast view of a single-partition AP (for DMA)."""
    return bass.AP(tensor=ap.tensor, offset=ap.offset, ap=[[0, parts]] + list(ap.ap)[1:])


def _ln_tile(nc, pool, out_ap, in_ap, eps_tile, g_bc, beta_bc):
    """LayerNorm over the free dim of a [128, H] tile: out = (x-m)/sqrt(v+eps)*g+b."""
    stats = pool.tile([128, nc.vector.BN_STATS_DIM], FP32, tag="ln_stats")
    nc.vector.bn_stats(out=stats[:], in_=in_ap)
    mv = pool.tile([128, nc.vector.BN_AGGR_DIM], FP32, tag="ln_mv")
    nc.vector.bn_aggr(out=mv[:], in_=stats[:])
    std = pool.tile([128, 1], FP32, tag="ln_std")
    nc.scalar.activation(out=std[:], in_=mv[:, 1:2], func=AF.Sqrt, bias=eps_tile[:])
    nc.vector.reciprocal(out=std[:], in_=std[:])
    tmp = pool.tile([128, H], FP32, tag="ln_tmp")
    nc.vector.tensor_scalar(
        out=tmp[:], in0=in_ap, scalar1=mv[:, 0:1], scalar2=std[:],
        op0=ALU.subtract, op1=ALU.mult,
    )
    nc.vector.tensor_mul(out=tmp[:], in0=tmp[:], in1=g_bc[:])
    nc.vector.tensor_add(out=out_ap, in0=tmp[:], in1=beta_bc[:])


def build_module():
    nc = bacc.Bacc(None)

    xb_d = nc.declare_dram_parameter("xb", [S, H], FP32, isOutput=False)
    xq_d = nc.declare_dram_parameter("xq", [SQ, H], FP32, isOutput=False)
    wq_d = nc.declare_dram_parameter("wq", [NH, H, H], FP32R, isOutput=False)
    bq_d = nc.declare_dram_parameter("bq", [NH, H], FP32, isOutput=False)
    wk_d = nc.declare_dram_parameter("wk", [NH, H, H], FP32R, isOutput=False)
    bk_d = nc.declare_dram_parameter("bk", [NH, H], FP32, isOutput=False)
    wv_d = nc.declare_dram_parameter("wv", [NH, H, H], FP32R, isOutput=False)
    bv_d = nc.declare_dram_parameter("bv", [NH, H], FP32, isOutput=False)
    wo_d = nc.declare_dram_parameter("wo", [NH * H, H], FP32R, isOutput=False)
    bo_d = nc.declare_dram_parameter("bo", [H], FP32, isOutput=False)
    w1_d = nc.declare_dram_parameter("w1", [H, F], FP32R, isOutput=False)
    b1_d = nc.declare_dram_parameter("b1", [F], FP32, isOutput=False)
    w2_d = nc.declare_dram_parameter("w2", [F, H], FP32R, isOutput=False)
    b2_d = nc.declare_dram_parameter("b2", [H], FP32, isOutput=False)
    g1_d = nc.declare_dram_parameter("g1", [H], FP32, isOutput=False)
    be1_d = nc.declare_dram_parameter("beta1", [H], FP32, isOutput=False)
    g2_d = nc.declare_dram_parameter("g2", [H], FP32, isOutput=False)
    be2_d = nc.declare_dram_parameter("beta2", [H], FP32, isOutput=False)
    out_d = nc.declare_dram_parameter("out", [SQ, H], FP32, isOutput=True)

    with tile.TileContext(nc) as tc, ExitStack() as ctx:
        singles = ctx.enter_context(tc.tile_pool(name="singles", bufs=1))
        work = ctx.enter_context(tc.tile_pool(name="work", bufs=3))

        # ---- constants / weights to SBUF ----
        ident = singles.tile([128, 128], FP32)
        make_identity(nc, ident[:])
        ones_st = singles.tile([128, 128], FP32)
        nc.vector.memset(ones_st[:], 1.0)
        ones128 = singles.tile([128, 128], FP32)  # all-ones lhsT: partition sums
        nc.vector.tensor_copy(out=_r(ones128[:]), in_=ones_st[:])
        eps_t = singles.tile([128, 1], FP32)
        nc.vector.memset(eps_t[:], LN_EPS)

        # ---- x into SBUF + transposes xT=[d, S], xqT=[d, SQ] ----
        xb_sb = singles.tile([128, TC, H], FP32)  # (s%128, sc, d)
        nc.sync.dma_start(out=xb_sb[:], in_=xb_d[:].rearrange("(sc p) d -> p sc d", p=128))
        xq_sb = singles.tile([128, SQ // 128, H], FP32)
        nc.sync.dma_start(out=xq_sb[:], in_=xq_d[:].rearrange("(sc p) d -> p sc d", p=128))
        xT = singles.tile([H, S], FP32)
        xqT = singles.tile([H, SQ], FP32)

        wq_sb = singles.tile([H, NH, H], FP32)    # (d, h, e)
        nc.sync.dma_start(out=_r(wq_sb[:]), in_=wq_d[:].rearrange("h d e -> d h e"))
        wk_sb = singles.tile([H, NH, H], FP32)
        nc.sync.dma_start(out=_r(wk_sb[:]), in_=wk_d[:].rearrange("h d e -> d h e"))
        wv_sb = singles.tile([H, NH, H], FP32)
        nc.sync.dma_start(out=_r(wv_sb[:]), in_=wv_d[:].rearrange("h d e -> d h e"))
        wo_sb = singles.tile([H, NH, H], FP32)    # (e, h, j)
        nc.sync.dma_start(out=_r(wo_sb[:]), in_=wo_d[:].rearrange("(h e) j -> e h j", h=NH))
        w1_sb = singles.tile([H, F], FP32)        # (d, f)
        nc.sync.dma_start(out=_r(w1_sb[:]), in_=w1_d[:])
        w2_sb = singles.tile([H, 2, H], FP32)     # (f%128, f//128, j)
        nc.sync.dma_start(out=_r(w2_sb[:]), in_=w2_d[:].rearrange("(c f) j -> f c j", c=2))

        bq_sb = singles.tile([H, NH], FP32)       # (e, h)
        nc.sync.dma_start(out=bq_sb[:], in_=bq_d[:].rearrange("h e -> e h"))
        bk_sb = singles.tile([H, NH], FP32)
        nc.sync.dma_start(out=bk_sb[:], in_=bk_d[:].rearrange("h e -> e h"))
        bv_sb = singles.tile([H, NH], FP32)
        nc.sync.dma_start(out=bv_sb[:], in_=bv_d[:].rearrange("h e -> e h"))
        bo_sb = singles.tile([H, 1], FP32)        # per-partition (j)
        nc.sync.dma_start(out=bo_sb[:], in_=bo_d[:].rearrange("(j o) -> j o", o=1))
        b1_sb = singles.tile([H, 2], FP32)        # (f%128, f//128)
        nc.sync.dma_start(out=b1_sb[:], in_=b1_d[:].rearrange("(c f) -> f c", c=2))
        b2_sb = singles.tile([H, 1], FP32)
        nc.sync.dma_start(out=b2_sb[:], in_=b2_d[:].rearrange("(j o) -> j o", o=1))

        g1_bc = singles.tile([128, H], FP32)      # free-dim vectors broadcast over partitions
        nc.sync.dma_start(out=g1_bc[:], in_=_bcast_ap(g1_d[:].rearrange("(o j) -> o j", o=1), 128))
        be1_bc = singles.tile([128, H], FP32)
        nc.sync.dma_start(out=be1_bc[:], in_=_bcast_ap(be1_d[:].rearrange("(o j) -> o j", o=1), 128))
        g2_bc = singles.tile([128, H], FP32)
        nc.sync.dma_start(out=g2_bc[:], in_=_bcast_ap(g2_d[:].rearrange("(o j) -> o j", o=1), 128))
        be2_bc = singles.tile([128, H], FP32)
        nc.sync.dma_start(out=be2_bc[:], in_=_bcast_ap(be2_d[:].rearrange("(o j) -> o j", o=1), 128))


        # PE matmuls (fused LDWEIGHTS) can carry only ONE semaphore wait in
        # codegen. Each dummy transpose below makes PE observe one DMA/engine
        # semaphore so no later matmul needs to wait on two at once; _zd()
        # writes a [1,1] dummy into a new PSUM pool's first tile so the
        # pool-transition (released-zone) dependency is absorbed there
        # instead of landing on a real matmul that also has a data wait.
        def _zd(tile_ap):
            nc.tensor.matmul(tile_ap[0:1, 0:1], ident[:, 0:1], ident[:, 0:1],
                             start=True, stop=True)

        with tc.tile_pool(name="abs_ps", bufs=7, space="PSUM") as abs_ps:
            for absorber in (
                ident[:], xb_sb[:, 0, :], wv_sb[:, 0, :].bitcast(FP32),
                wq_sb[:, 0, :].bitcast(FP32), wo_sb[:, 0, :].bitcast(FP32),
                w1_sb[:, 0:128].bitcast(FP32), w2_sb[:, 0, :].bitcast(FP32),
            ):
                pt = abs_ps.tile([128, 128], FP32, tag="abs")
                nc.tensor.transpose(pt[:], absorber, ident[:])

        with tc.tile_pool(name="tp_ps", bufs=2, space="PSUM") as tp_ps:
            for sc in range(SQ // 128):
                pt = tp_ps.tile([128, 128], FP32, tag="tp")
                if sc == 0:
                    _zd(pt)
                nc.tensor.transpose(pt[:], xq_sb[:, sc, :], ident[:])
                nc.vector.tensor_copy(out=_r(xqT[:, sc * 128:(sc + 1) * 128]), in_=pt[:])
            for sc in range(TC):
                pt = tp_ps.tile([128, 128], FP32, tag="tp")
                nc.tensor.transpose(pt[:], xb_sb[:, sc, :], ident[:])
                nc.vector.tensor_copy(out=_r(xT[:, sc * 128:(sc + 1) * 128]), in_=pt[:])

        # ---- V for all heads: v_sb[t%128, tc, h, e] = (x @ Wv)[t, (h e)] ----
        v_sb = singles.tile([128, TC, NH, H], FP32)
        with tc.tile_pool(name="v_ps", bufs=2, space="PSUM") as v_ps:
            for tcc in range(TC):
                vp = v_ps.tile([128, NH * H], FP32, tag="v")
                if tcc == 0:
                    _zd(vp)
                for half in range(2):
                    nc.tensor.matmul(
                        vp[:, half * 512:(half + 1) * 512],
                        _r(xT[:, tcc * 128:(tcc + 1) * 128]),
                        _r(wv_sb[:, half * 4:(half + 1) * 4, :]),
                        start=True, stop=True,
                    )
                nc.vector.tensor_copy(out=_r(v_sb[:, tcc, :, :]), in_=vp[:])

        # ---- attention head loop ----
        kt_pool = ctx.enter_context(tc.tile_pool(name="kt", bufs=2))
        qt_pool = ctx.enter_context(tc.tile_pool(name="qt", bufs=2))
        pt_pool = ctx.enter_context(tc.tile_pool(name="pt", bufs=3))
        ot_pool = ctx.enter_context(tc.tile_pool(name="ot", bufs=2))

        yT_sb = singles.tile([H, SQ], FP32)  # attention block output (pre-LN), [j, s]

        with (
            tc.tile_pool(name="s_ps", bufs=2, space="PSUM") as s_ps,
            tc.tile_pool(name="o_ps", bufs=2, space="PSUM") as o_ps,
            tc.tile_pool(name="d_ps", bufs=1, space="PSUM") as d_ps,
            tc.tile_pool(name="y_ps", bufs=1, space="PSUM") as y_ps,
        ):
            y_acc = y_ps.tile([H, SQ], FP32)
            _zd(y_acc)

            # o^T = o_acc / denom + bv (softmax rows sum to 1). Emitted at the
            # START of the next head so the DVE chain runs while PE streams the
            # next head's matmuls, and the Wo matmul is emitted AFTER that
            # head's t-loop so in-order PE never stalls on it.
            def _finalize_dve(hp, o_p, d_p):
                rec_bc = ot_pool.tile([128, SQ], FP32, tag="rec")
                nc.vector.reciprocal(out=rec_bc[:], in_=d_p[:])
                oT = ot_pool.tile([H, SQ], FP32, tag="oT")
                nc.vector.tensor_mul(out=_r(oT[:]), in0=o_p[:], in1=rec_bc[:])
                nc.vector.tensor_scalar_add(out=_r(oT[:]), in0=oT[:],
                                            scalar1=bv_sb[:, hp:hp + 1])
                return oT

            prev = None  # (h, o_acc, d_acc)
            for h in range(NH):
                oT_prev = _finalize_dve(*prev) if prev is not None else None

                # K^T[e, t] and Q^T[e, s] with biases (1/sqrt(H) folded into Q)
                kt = kt_pool.tile([H, S], FP32, tag="kt")
                for i in range(S // 512):
                    kp = s_ps.tile([128, 1024], FP32, tag="s")
                    if h == 0 and i == 0:
                        _zd(kp)
                    nc.tensor.matmul(
                        kp[:, 0:512], _r(wk_sb[:, h, :]), _r(xT[:, i * 512:(i + 1) * 512]),
                        start=True, stop=True,
                    )
                    nc.vector.tensor_scalar_add(
                        out=_r(kt[:, i * 512:(i + 1) * 512]), in0=kp[:, 0:512],
                        scalar1=bk_sb[:, h:h + 1],
                    )
                qt = qt_pool.tile([H, SQ], FP32, tag="qt")
                qp = s_ps.tile([128, 1024], FP32, tag="s")
                nc.tensor.matmul(qp[:, 0:512], _r(wq_sb[:, h, :]), _r(xqT[:]),
                                 start=True, stop=True)
                nc.vector.tensor_scalar(
                    out=_r(qt[:]), in0=qp[:, 0:512], scalar1=bq_sb[:, h:h + 1],
                    scalar2=1.0 / math.sqrt(H), op0=ALU.add, op1=ALU.mult,
                )

                o_acc = o_ps.tile([H, SQ], FP32, tag="o")
                d_acc = d_ps.tile([128, SQ], FP32, tag="d")
                if h == 0:
                    _zd(o_acc)
                    _zd(d_acc)

                for g in range(TC // 2):  # pairs of t-chunks share one 2-bank psum tile
                    sp = s_ps.tile([128, 1024], FP32, tag="s")
                    for j in range(2):
                        tcc = 2 * g + j
                        nc.tensor.matmul(
                            sp[:, j * 512:(j + 1) * 512],
                            _r(kt[:, tcc * 128:(tcc + 1) * 128]), _r(qt[:]),
                            start=True, stop=True,
                        )
                    pt = pt_pool.tile([128, 1024], FP32, tag="pt")
                    nc.scalar.activation(out=_r(pt[:]), in_=sp[:], func=AF.Exp)
                    for j in range(2):
                        tcc = 2 * g + j
                        nc.tensor.matmul(
                            d_acc[:], _r(ones128[:]), _r(pt[:, j * 512:(j + 1) * 512]),
                            start=(tcc == 0), stop=(tcc == TC - 1),
                        )
                        nc.tensor.matmul(
                            o_acc[:], _r(v_sb[:, tcc, h, :]), _r(pt[:, j * 512:(j + 1) * 512]),
                            start=(tcc == 0), stop=(tcc == TC - 1),
                        )

                if prev is not None:
                    nc.tensor.matmul(y_acc[:], _r(wo_sb[:, prev[0], :]), _r(oT_prev[:]),
                                     start=(prev[0] == 0), stop=False)
                prev = (h, o_acc, d_acc)

            oT_last = _finalize_dve(*prev)
            nc.tensor.matmul(y_acc[:], _r(wo_sb[:, NH - 1, :]), _r(oT_last[:]),
                             start=False, stop=True)
            nc.vector.tensor_scalar_add(out=yT_sb[:], in0=y_acc[:], scalar1=bo_sb[:])

        # ---- epilogue: transpose y, LN1, FFN (transposed), residual, LN2 ----
        y1_sb = singles.tile([128, SQ // 128, H], FP32)   # LN1 output, natural (s, j)
        y1T = singles.tile([H, SQ], FP32)                 # LN1 output, [d, s]
        out_sb = singles.tile([128, SQ // 128, H], FP32)

        with (
            tc.tile_pool(name="e_ps", bufs=2, space="PSUM") as e_ps,
            tc.tile_pool(name="u_ps", bufs=2, space="PSUM") as u_ps,
            tc.tile_pool(name="z_ps", bufs=1, space="PSUM") as z_ps,
        ):
            for sc in range(SQ // 128):
                yp = e_ps.tile([128, 128], FP32, tag="e")
                if sc == 0:
                    _zd(yp)
                nc.tensor.transpose(yp[:], yT_sb[:, sc * 128:(sc + 1) * 128], ident[:])
                _ln_tile(nc, work, y1_sb[:, sc, :], yp[:], eps_t, g1_bc, be1_bc)
            for sc in range(SQ // 128):
                yp = e_ps.tile([128, 128], FP32, tag="e")
                nc.tensor.transpose(yp[:], y1_sb[:, sc, :], ident[:])
                nc.vector.tensor_copy(out=_r(y1T[:, sc * 128:(sc + 1) * 128]), in_=yp[:])

            # u^T[f, s] = relu(W1^T y1 + b1), f in two 128-chunks
            uT = work.tile([H, 2, SQ], FP32, tag="uT")
            for fc in range(2):
                up = u_ps.tile([128, SQ], FP32, tag="u")
                if fc == 0:
                    _zd(up)
                nc.tensor.matmul(up[:], _r(w1_sb[:, fc * 128:(fc + 1) * 128]), _r(y1T[:]),
                                 start=True, stop=True)
                nc.scalar.activation(out=_r(uT[:, fc, :]), in_=up[:], func=AF.Relu,
                                     bias=b1_sb[:, fc:fc + 1])
            # z^T[j, s] = relu(W2^T u + b2)
            zp = z_ps.tile([H, SQ], FP32, tag="z")
            _zd(zp)
            for fc in range(2):
                nc.tensor.matmul(zp[:], _r(w2_sb[:, fc, :]), _r(uT[:, fc, :]),
                                 start=(fc == 0), stop=(fc == 1))
            zT = work.tile([H, SQ], FP32, tag="zT")
            nc.scalar.activation(out=zT[:], in_=zp[:], func=AF.Relu, bias=b2_sb[:])

            # residual + LN2, back in natural layout
            for sc in range(SQ // 128):
                rp = e_ps.tile([128, 128], FP32, tag="e")
                nc.tensor.transpose(rp[:], zT[:, sc * 128:(sc + 1) * 128], ident[:])
                r_sb = work.tile([128, H], FP32, tag="r_sb")
                nc.vector.tensor_add(out=r_sb[:], in0=rp[:], in1=y1_sb[:, sc, :])
                _ln_tile(nc, work, out_sb[:, sc, :], r_sb[:], eps_t, g2_bc, be2_bc)

        nc.sync.dma_start(out=out_d[:].rearrange("(sc p) j -> p sc j", p=128), in_=out_sb[:])

    nc.finalize()
    return nc


_CACHE: dict = {}


def _get_nc():
    if "nc" not in _CACHE:
        _CACHE["nc"] = build_module()
    return _CACHE["nc"]


def _in_maps(inputs):
    f32 = lambda a: np.ascontiguousarray(np.asarray(a), dtype=np.float32)
    x = f32(inputs["x"])
    shared = {
        "wq": f32(inputs["Wq"]), "bq": f32(inputs["bq"]),
        "wk": f32(inputs["Wk"]), "bk": f32(inputs["bk"]),
        "wv": f32(inputs["Wv"]), "bv": f32(inputs["bv"]),
        "wo": f32(inputs["Wo"]), "bo": f32(inputs["bo"]),
        "w1": f32(inputs["W1"]), "b1": f32(inputs["b1"]),
        "w2": f32(inputs["W2"]), "b2": f32(inputs["b2"]),
        "g1": f32(inputs["g1"]), "beta1": f32(inputs["beta1"]),
        "g2": f32(inputs["g2"]), "beta2": f32(inputs["beta2"]),
    }
    maps = []
    for c in range(NCORES):
        b, qi = divmod(c, NCORES // B)
        q0 = qi * SQ
        maps.append({
            "xb": np.ascontiguousarray(x[b]),
            "xq": np.ascontiguousarray(x[b, q0:q0 + SQ]),
            **shared,
        })
    return maps


def run(inputs, **kwargs):
    nc = _get_nc()
    res = run_bass_kernel_spmd(nc, _in_maps(inputs), core_ids=list(range(NCORES)), **kwargs)
    parts = [res.results[c]["out"] for c in range(NCORES)]
    y = np.concatenate(parts, axis=0).reshape(B, S, H).astype(np.float32)
    return y, res


def kernel(**inputs) -> np.ndarray:
    y, _ = run(inputs)
    return y


# revision 15
# speedup vs baseline: 1.1503x; 1.1215x over previous
"""Trainium2 Bass kernel for nn_EncodingLayer (dense transformer encoder layer).

Reference computation (B=2, S=2048, H=128, NH=8):
    Q/K/V = per-head full-dim projections of x, scores = QK^T/sqrt(H),
    A = softmax(scores), o = A@V, concat heads, y = o@Wo+bo,
    y = LN1(y), f = relu(relu(y@W1+b1)@W2+b2), out = LN2(y+f).

Sharding: data-parallel over query rows. Core c (of 8) owns batch b=c//4 and
query rows q0=(c%4)*512 .. q0+512 of that batch. Each core computes K/V for
its full batch (4x replicated compute, tiny) and the full epilogue for its
512 rows. No collectives; host concatenates the 8 [512,128] slices.

Within a core the attention runs in "transposed score" layout:
    QT/KT = [e, s] via PE, scores^T[t,s] chunks on PE (fp32r, full rate),
    P^T = exp(scores^T) on ACT straight out of PSUM, o^T accumulated on PE
    with V[t,e] chunks as stationary, softmax denominator via ones-vector
    matmul (sum over t = partition dim), division applied to o^T (tiny).
Since |scores| < ~0.4 for this problem scale, softmax without max-subtraction
is numerically exact; bv folds into o^T after division because softmax rows
sum to one.
"""

import math
import numpy as np
from contextlib import ExitStack

import concourse.bass as bass
import concourse.bacc as bacc
import concourse.mybir as mybir
import concourse.tile as tile
from concourse.bass_utils import run_bass_kernel_spmd
from concourse.masks import make_identity

B, S, H, NH = 2, 2048, 128, 8
F = 2 * H                      # FFN hidden dim (256)
NCORES = 8
SQ = (B * S) // NCORES         # 512 query rows per core
TC = S // 128                  # 16 key/value chunks of 128
LN_EPS = 1e-5
FP32 = mybir.dt.float32
FP32R = mybir.dt.float32r
BF16 = mybir.dt.bfloat16
AF = mybir.ActivationFunctionType
ALU = mybir.AluOpType


def _r(ap):
    return ap.bitcast(FP32R)


def _bcast_ap(ap, parts):
    """Partition-broadcast view of a single-partition AP (for DMA)."""
    return bass.AP(tensor=ap.tensor, offset=ap.offset, ap=[[0, parts]] + list(ap.ap)[1:])


def _ln_tile(nc, pool, out_ap, in_ap, eps_tile, g_bc, beta_bc):
    """LayerNorm over the free dim of a [128, H] tile: out = (x-m)/sqrt(v+eps)*g+b."""
    stats = pool.tile([128, nc.vector.BN_STATS_DIM], FP32, tag="ln_stats")
    nc.vector.bn_stats(out=stats[:], in_=in_ap)
    mv = pool.tile([128, nc.vector.BN_AGGR_DIM], FP32, tag="ln_mv")
    nc.vector.bn_aggr(out=mv[:], in_=stats[:])
    std = pool.tile([128, 1], FP32, tag="ln_std")
    nc.scalar.activation(out=std[:], in_=mv[:, 1:2], func=AF.Sqrt, bias=eps_tile[:])
    nc.vector.reciprocal(out=std[:], in_=std[:])
    tmp = pool.tile([128, H], FP32, tag="ln_tmp")
    nc.vector.tensor_scalar(
        out=tmp[:], in0=in_ap, scalar1=mv[:, 0:1], scalar2=std[:],
        op0=ALU.subtract, op1=ALU.mult,
    )
    nc.vector.tensor_mul(out=tmp[:], in0=tmp[:], in1=g_bc[:])
    nc.vector.tensor_add(out=out_ap, in0=tmp[:], in1=beta_bc[:])


def build_module():
    nc = bacc.Bacc(None)

    xb_d = nc.declare_dram_parameter("xb", [S, H], FP32, isOutput=False)
    xq_d = nc.declare_dram_parameter("xq", [SQ, H], FP32, isOutput=False)
    wq_d = nc.declare_dram_parameter("wq", [NH, H, H], FP32R, isOutput=False)
    bq_d = nc.declare_dram_parameter("bq", [NH, H], FP32, isOutput=False)
    wk_d = nc.declare_dram_parameter("wk", [NH, H, H], FP32R, isOutput=False)
    bk_d = nc.declare_dram_parameter("bk", [NH, H], FP32, isOutput=False)
    wv_d = nc.declare_dram_parameter("wv", [NH, H, H], FP32R, isOutput=False)
    bv_d = nc.declare_dram_parameter("bv", [NH, H], FP32, isOutput=False)
    wo_d = nc.declare_dram_parameter("wo", [NH * H, H], FP32R, isOutput=False)
    bo_d = nc.declare_dram_parameter("bo", [H], FP32, isOutput=False)
    w1_d = nc.declare_dram_parameter("w1", [H, F], FP32R, isOutput=False)
    b1_d = nc.declare_dram_parameter("b1", [F], FP32, isOutput=False)
    w2_d = nc.declare_dram_parameter("w2", [F, H], FP32R, isOutput=False)
    b2_d = nc.declare_dram_parameter("b2", [H], FP32, isOutput=False)
    g1_d = nc.declare_dram_parameter("g1", [H], FP32, isOutput=False)
    be1_d = nc.declare_dram_parameter("beta1", [H], FP32, isOutput=False)
    g2_d = nc.declare_dram_parameter("g2", [H], FP32, isOutput=False)
    be2_d = nc.declare_dram_parameter("beta2", [H], FP32, isOutput=False)
    out_d = nc.declare_dram_parameter("out", [SQ, H], FP32, isOutput=True)

    with tile.TileContext(nc) as tc, ExitStack() as ctx:
        singles = ctx.enter_context(tc.tile_pool(name="singles", bufs=1))
        work = ctx.enter_context(tc.tile_pool(name="work", bufs=3))

        # ---- constants / weights to SBUF ----
        ident = singles.tile([128, 128], FP32)
        make_identity(nc, ident[:])
        ones_st = singles.tile([128, 128], FP32)
        nc.vector.memset(ones_st[:], 1.0)
        ones128 = singles.tile([128, 128], FP32)  # all-ones lhsT: partition sums
        nc.vector.tensor_copy(out=_r(ones128[:]), in_=ones_st[:])
        eps_t = singles.tile([128, 1], FP32)
        nc.vector.memset(eps_t[:], LN_EPS)

        # ---- x into SBUF + transposes xT=[d, S], xqT=[d, SQ] ----
        xb_sb = singles.tile([128, TC, H], FP32)  # (s%128, sc, d)
        xb_r = xb_d[:].rearrange("(sc p) d -> p sc d", p=128)
        for q in range(4):
            nc.sync.dma_start(out=xb_sb[:, 4 * q:4 * (q + 1), :], in_=xb_r[:, 4 * q:4 * (q + 1), :])
        xq_sb = singles.tile([128, SQ // 128, H], FP32)
        nc.sync.dma_start(out=xq_sb[:], in_=xq_d[:].rearrange("(sc p) d -> p sc d", p=128))
        xT = singles.tile([H, S], FP32)
        xqT = singles.tile([H, SQ], FP32)

        wq_sb = singles.tile([H, NH, H], FP32)    # (d, h, e)
        nc.sync.dma_start(out=_r(wq_sb[:]), in_=wq_d[:].rearrange("h d e -> d h e"))
        wk_sb = singles.tile([H, NH, H], FP32)
        nc.sync.dma_start(out=_r(wk_sb[:]), in_=wk_d[:].rearrange("h d e -> d h e"))
        wv_sb = singles.tile([H, NH, H], FP32)
        nc.sync.dma_start(out=_r(wv_sb[:]), in_=wv_d[:].rearrange("h d e -> d h e"))
        wo_sb = singles.tile([H, NH, H], FP32)    # (e, h, j)
        nc.sync.dma_start(out=_r(wo_sb[:]), in_=wo_d[:].rearrange("(h e) j -> e h j", h=NH))
        w1_sb = singles.tile([H, F], FP32)        # (d, f)
        nc.sync.dma_start(out=_r(w1_sb[:]), in_=w1_d[:])
        w2_sb = singles.tile([H, 2, H], FP32)     # (f%128, f//128, j)
        nc.sync.dma_start(out=_r(w2_sb[:]), in_=w2_d[:].rearrange("(c f) j -> f c j", c=2))

        bq_sb = singles.tile([H, NH], FP32)       # (e, h)
        nc.sync.dma_start(out=bq_sb[:], in_=bq_d[:].rearrange("h e -> e h"))
        bk_sb = singles.tile([H, NH], FP32)
        nc.sync.dma_start(out=bk_sb[:], in_=bk_d[:].rearrange("h e -> e h"))
        bv_sb = singles.tile([H, NH], FP32)
        nc.sync.dma_start(out=bv_sb[:], in_=bv_d[:].rearrange("h e -> e h"))
        bo_sb = singles.tile([H, 1], FP32)        # per-partition (j)
        nc.sync.dma_start(out=bo_sb[:], in_=bo_d[:].rearrange("(j o) -> j o", o=1))
        b1_sb = singles.tile([H, 2], FP32)        # (f%128, f//128)
        nc.sync.dma_start(out=b1_sb[:], in_=b1_d[:].rearrange("(c f) -> f c", c=2))
        b2_sb = singles.tile([H, 1], FP32)
        nc.sync.dma_start(out=b2_sb[:], in_=b2_d[:].rearrange("(j o) -> j o", o=1))

        g1_bc = singles.tile([128, H], FP32)      # free-dim vectors broadcast over partitions
        nc.sync.dma_start(out=g1_bc[:], in_=_bcast_ap(g1_d[:].rearrange("(o j) -> o j", o=1), 128))
        be1_bc = singles.tile([128, H], FP32)
        nc.sync.dma_start(out=be1_bc[:], in_=_bcast_ap(be1_d[:].rearrange("(o j) -> o j", o=1), 128))
        g2_bc = singles.tile([128, H], FP32)
        nc.sync.dma_start(out=g2_bc[:], in_=_bcast_ap(g2_d[:].rearrange("(o j) -> o j", o=1), 128))
        be2_bc = singles.tile([128, H], FP32)
        nc.sync.dma_start(out=be2_bc[:], in_=_bcast_ap(be2_d[:].rearrange("(o j) -> o j", o=1), 128))


        # PE matmuls (fused LDWEIGHTS) can carry only ONE semaphore wait in
        # codegen. Each dummy transpose below makes PE observe one DMA/engine
        # semaphore so no later matmul needs to wait on two at once; _zd()
        # writes a [1,1] dummy into a new PSUM pool's first tile so the
        # pool-transition (released-zone) dependency is absorbed there
        # instead of landing on a real matmul that also has a data wait.
        def _zd(tile_ap):
            nc.tensor.matmul(tile_ap[0:1, 0:1], ident[:, 0:1], ident[:, 0:1],
                             start=True, stop=True)

        with tc.tile_pool(name="abs_ps", bufs=7, space="PSUM") as abs_ps:
            for absorber in (
                ident[:], xb_sb[:, 0, :], wv_sb[:, 0, :].bitcast(FP32),
                wq_sb[:, 0, :].bitcast(FP32), wo_sb[:, 0, :].bitcast(FP32),
                w1_sb[:, 0:128].bitcast(FP32), w2_sb[:, 0, :].bitcast(FP32),
            ):
                pt = abs_ps.tile([128, 128], FP32, tag="abs")
                nc.tensor.transpose(pt[:], absorber, ident[:])

        with tc.tile_pool(name="tp_ps", bufs=2, space="PSUM") as tp_ps:
            for sc in range(SQ // 128):
                pt = tp_ps.tile([128, 128], FP32, tag="tp")
                if sc == 0:
                    _zd(pt)
                nc.tensor.transpose(pt[:], xq_sb[:, sc, :], ident[:])
                nc.vector.tensor_copy(out=_r(xqT[:, sc * 128:(sc + 1) * 128]), in_=pt[:])
            for sc in range(TC):
                pt = tp_ps.tile([128, 128], FP32, tag="tp")
                nc.tensor.transpose(pt[:], xb_sb[:, sc, :], ident[:])
                nc.vector.tensor_copy(out=_r(xT[:, sc * 128:(sc + 1) * 128]), in_=pt[:])

        # ---- V for all heads: v_sb[t%128, tc, h, e] = (x @ Wv)[t, (h e)] ----
        v_sb = singles.tile([128, TC, NH, H], FP32)
        with tc.tile_pool(name="v_ps", bufs=2, space="PSUM") as v_ps:
            for tcc in range(TC):
                vp = v_ps.tile([128, NH * H], FP32, tag="v")
                if tcc == 0:
                    _zd(vp)
                for half in range(2):
                    nc.tensor.matmul(
                        vp[:, half * 512:(half + 1) * 512],
                        _r(xT[:, tcc * 128:(tcc + 1) * 128]),
                        _r(wv_sb[:, half * 4:(half + 1) * 4, :]),
                        start=True, stop=True,
                    )
                nc.vector.tensor_copy(out=_r(v_sb[:, tcc, :, :]), in_=vp[:])

        # ---- attention head loop ----
        kt_pool = ctx.enter_context(tc.tile_pool(name="kt", bufs=2))
        qt_pool = ctx.enter_context(tc.tile_pool(name="qt", bufs=2))
        pt_pool = ctx.enter_context(tc.tile_pool(name="pt", bufs=3))
        ot_pool = ctx.enter_context(tc.tile_pool(name="ot", bufs=2))

        yT_sb = singles.tile([H, SQ], FP32)  # attention block output (pre-LN), [j, s]

        with (
            tc.tile_pool(name="s_ps", bufs=2, space="PSUM") as s_ps,
            tc.tile_pool(name="o_ps", bufs=2, space="PSUM") as o_ps,
            tc.tile_pool(name="d_ps", bufs=1, space="PSUM") as d_ps,
            tc.tile_pool(name="y_ps", bufs=1, space="PSUM") as y_ps,
        ):
            y_acc = y_ps.tile([H, SQ], FP32)
            _zd(y_acc)

            # o^T = o_acc / denom + bv (softmax rows sum to 1). Emitted at the
            # START of the next head so the DVE chain runs while PE streams the
            # next head's matmuls, and the Wo matmul is emitted AFTER that
            # head's t-loop so in-order PE never stalls on it.
            def _finalize_dve(hp, o_p, d_p):
                rec_bc = ot_pool.tile([128, SQ], FP32, tag="rec")
                scr = ot_pool.tile([128, SQ], FP32, tag="rec_scr")
                nc.vector.reciprocal_approx_accurate(out=rec_bc[:], in_=d_p[:], scratch=scr[:])
                oT = ot_pool.tile([H, SQ], FP32, tag="oT")
                nc.vector.tensor_mul(out=_r(oT[:]), in0=o_p[:], in1=rec_bc[:])
                nc.vector.tensor_scalar_add(out=_r(oT[:]), in0=oT[:],
                                            scalar1=bv_sb[:, hp:hp + 1])
                return oT

            prev = None  # (h, o_acc, d_acc)
            for h in range(NH):
                oT_prev = None

                # K^T[e, t] and Q^T[e, s] with biases (1/sqrt(H) folded into Q)
                kt = kt_pool.tile([H, S], BF16, tag="kt")
                for i in range(S // 512):
                    kp = s_ps.tile([128, 1024], FP32, tag="s")
                    if h == 0 and i == 0:
                        _zd(kp)
                    nc.tensor.matmul(
                        kp[:, 0:512], _r(wk_sb[:, h, :]), _r(xT[:, i * 512:(i + 1) * 512]),
                        start=True, stop=True,
                    )
                    nc.vector.tensor_scalar_add(
                        out=kt[:, i * 512:(i + 1) * 512], in0=kp[:, 0:512],
                        scalar1=bk_sb[:, h:h + 1],
                    )
                qt = qt_pool.tile([H, SQ], BF16, tag="qt")
                qp = s_ps.tile([128, 1024], FP32, tag="s")
                nc.tensor.matmul(qp[:, 0:512], _r(wq_sb[:, h, :]), _r(xqT[:]),
                                 start=True, stop=True)
                nc.vector.tensor_scalar(
                    out=qt[:], in0=qp[:, 0:512], scalar1=bq_sb[:, h:h + 1],
                    scalar2=1.0 / math.sqrt(H), op0=ALU.add, op1=ALU.mult,
                )

                o_acc = o_ps.tile([H, SQ], FP32, tag="o")
                d_acc = d_ps.tile([128, SQ], FP32, tag="d")
                if h == 0:
                    _zd(o_acc)
                    _zd(d_acc)

                for g in range(TC // 2):  # pairs of t-chunks share one 2-bank psum tile
                    if g == 2 and prev is not None:
                        oT_prev = _finalize_dve(*prev)
                    sp = s_ps.tile([128, 1024], FP32, tag="s")
                    for j in range(2):
                        tcc = 2 * g + j
                        nc.tensor.matmul(
                            sp[:, j * 512:(j + 1) * 512],
                            kt[:, tcc * 128:(tcc + 1) * 128], qt[:],
                            start=True, stop=True,
                        )
                    pt = pt_pool.tile([128, 1024], FP32, tag="pt")
                    nc.scalar.activation(out=_r(pt[:]), in_=sp[:], func=AF.Exp)
                    for j in range(2):
                        tcc = 2 * g + j
                        nc.tensor.matmul(
                            d_acc[:], _r(ones128[:]), _r(pt[:, j * 512:(j + 1) * 512]),
                            start=(tcc == 0), stop=(tcc == TC - 1),
                        )
                        nc.tensor.matmul(
                            o_acc[:], _r(v_sb[:, tcc, h, :]), _r(pt[:, j * 512:(j + 1) * 512]),
                            start=(tcc == 0), stop=(tcc == TC - 1),
                        )

                if prev is not None:
                    nc.tensor.matmul(y_acc[:], _r(wo_sb[:, prev[0], :]), _r(oT_prev[:]),
                                     start=(prev[0] == 0), stop=False)
                prev = (h, o_acc, d_acc)

            oT_last = _finalize_dve(*prev)
            nc.tensor.matmul(y_acc[:], _r(wo_sb[:, NH - 1, :]), _r(oT_last[:]),
                             start=False, stop=True)
            nc.vector.tensor_scalar_add(out=yT_sb[:], in0=y_acc[:], scalar1=bo_sb[:])

        # ---- epilogue: transpose y, LN1, FFN (transposed), residual, LN2 ----
        y1_sb = singles.tile([128, SQ // 128, H], FP32)   # LN1 output, natural (s, j)
        y1T = singles.tile([H, SQ], FP32)                 # LN1 output, [d, s]
        out_sb = singles.tile([128, SQ // 128, H], FP32)

        with (
            tc.tile_pool(name="e_ps", bufs=2, space="PSUM") as e_ps,
            tc.tile_pool(name="u_ps", bufs=2, space="PSUM") as u_ps,
            tc.tile_pool(name="z_ps", bufs=1, space="PSUM") as z_ps,
        ):
            for sc in range(SQ // 128):
                yp = e_ps.tile([128, 128], FP32, tag="e")
                if sc == 0:
                    _zd(yp)
                nc.tensor.transpose(yp[:], yT_sb[:, sc * 128:(sc + 1) * 128], ident[:])
                _ln_tile(nc, work, y1_sb[:, sc, :], yp[:], eps_t, g1_bc, be1_bc)
            for sc in range(SQ // 128):
                yp = e_ps.tile([128, 128], FP32, tag="e")
                nc.tensor.transpose(yp[:], y1_sb[:, sc, :], ident[:])
                nc.vector.tensor_copy(out=_r(y1T[:, sc * 128:(sc + 1) * 128]), in_=yp[:])

            # u^T[f, s] = relu(W1^T y1 + b1), f in two 128-chunks
            uT = work.tile([H, 2, SQ], FP32, tag="uT")
            for fc in range(2):
                up = u_ps.tile([128, SQ], FP32, tag="u")
                if fc == 0:
                    _zd(up)
                nc.tensor.matmul(up[:], _r(w1_sb[:, fc * 128:(fc + 1) * 128]), _r(y1T[:]),
                                 start=True, stop=True)
                nc.scalar.activation(out=_r(uT[:, fc, :]), in_=up[:], func=AF.Relu,
                                     bias=b1_sb[:, fc:fc + 1])
            # z^T[j, s] = relu(W2^T u + b2)
            zp = z_ps.tile([H, SQ], FP32, tag="z")
            _zd(zp)
            for fc in range(2):
                nc.tensor.matmul(zp[:], _r(w2_sb[:, fc, :]), _r(uT[:, fc, :]),
                                 start=(fc == 0), stop=(fc == 1))
            zT = work.tile([H, SQ], FP32, tag="zT")
            nc.scalar.activation(out=zT[:], in_=zp[:], func=AF.Relu, bias=b2_sb[:])

            # residual + LN2, back in natural layout
            for sc in range(SQ // 128):
                rp = e_ps.tile([128, 128], FP32, tag="e")
                nc.tensor.transpose(rp[:], zT[:, sc * 128:(sc + 1) * 128], ident[:])
                r_sb = work.tile([128, H], FP32, tag="r_sb")
                nc.vector.tensor_add(out=r_sb[:], in0=rp[:], in1=y1_sb[:, sc, :])
                _ln_tile(nc, work, out_sb[:, sc, :], r_sb[:], eps_t, g2_bc, be2_bc)

        nc.sync.dma_start(out=out_d[:].rearrange("(sc p) j -> p sc j", p=128), in_=out_sb[:])

    nc.finalize()
    return nc


_CACHE: dict = {}


def _get_nc():
    if "nc" not in _CACHE:
        _CACHE["nc"] = build_module()
    return _CACHE["nc"]


def _in_maps(inputs):
    f32 = lambda a: np.ascontiguousarray(np.asarray(a), dtype=np.float32)
    x = f32(inputs["x"])
    shared = {
        "wq": f32(inputs["Wq"]), "bq": f32(inputs["bq"]),
        "wk": f32(inputs["Wk"]), "bk": f32(inputs["bk"]),
        "wv": f32(inputs["Wv"]), "bv": f32(inputs["bv"]),
        "wo": f32(inputs["Wo"]), "bo": f32(inputs["bo"]),
        "w1": f32(inputs["W1"]), "b1": f32(inputs["b1"]),
        "w2": f32(inputs["W2"]), "b2": f32(inputs["b2"]),
        "g1": f32(inputs["g1"]), "beta1": f32(inputs["beta1"]),
        "g2": f32(inputs["g2"]), "beta2": f32(inputs["beta2"]),
    }
    maps = []
    for c in range(NCORES):
        b, qi = divmod(c, NCORES // B)
        q0 = qi * SQ
        maps.append({
            "xb": np.ascontiguousarray(x[b]),
            "xq": np.ascontiguousarray(x[b, q0:q0 + SQ]),
            **shared,
        })
    return maps


def run(inputs, **kwargs):
    nc = _get_nc()
    res = run_bass_kernel_spmd(nc, _in_maps(inputs), core_ids=list(range(NCORES)), **kwargs)
    parts = [res.results[c]["out"] for c in range(NCORES)]
    y = np.concatenate(parts, axis=0).reshape(B, S, H).astype(np.float32)
    return y, res


def kernel(**inputs) -> np.ndarray:
    y, _ = run(inputs)
    return y
